# revision 58
# baseline (speedup 1.0000x reference)
"""Trainium2 Bass kernel for nn_Attn_17738214933129.

Dense transformer attention block:
  Q/K/V projections from n_loc=2048 -> feat=512 (8 heads x 64),
  structural-bias softmax added to scaled QK^T scores, softmax, PV,
  output projection back to n_loc=2048.

Sharding: data-parallel over batch (16 -> 2 per core) across 8 NeuronCores,
weights replicated, no collectives.

Layout strategy (per core, rows = 2*512 = 1024):
  - q/k are uploaded pre-transposed/pre-tiled in fp8e4; Q/K projections run
    as fp8 DoubleRow matmuls (2 k-subtiles per instruction, 2x PE rate)
    with weights host-scaled by 64 to stay in fp8 normal range; the
    PSUM->SBUF copy descales (and folds the 1/DH for Q).
  - v is bf16; V is projected directly into [rows, feat] layout (input
    slice as the stationary operand), so no PE transpose pass is needed.
  - Attention uses the TRANSPOSED score layout: ST[k, q] = (QK^T)^T per
    head, so exp(ST)*esmT feeds the PV matmul directly (contraction over
    keys on partitions) -- no on-chip transposes at all.  The structural
    softmax is precomputed on host and uploaded as esmT = exp(sm^T) bf16;
    the multiply runs on DVE in fast 2-byte mode.
  - V carries 64 ones-columns per head ([V_h | 1...1]), so PV output rows
    0-63 hold the softmax row-sum replicated 64x: the reciprocal runs wide
    on DVE and its output is already broadcast for the normalization,
    which fuses into the PSUM->SBUF copy as a tensor-tensor multiply.
  - All PSUM work uses 2-bank pair tiles [128, 2, 512] so elementwise ops
    (exp / esm-mult / copies / reciprocal) amortize their fixed per-op
    overhead over 1024 columns.
"""

import sys

import numpy as np

try:
    import concourse.bass as bass  # noqa: F401
except Exception:  # pragma: no cover - path fallback
    sys.path.insert(0, "/opt/trn_rl_repo")

import ml_dtypes

import concourse.bacc as bacc
import concourse.tile as tile
from concourse import mybir
from concourse.bass_utils import run_bass_kernel_spmd

BF16 = mybir.dt.bfloat16
FP8 = mybir.dt.float8e4
F32 = mybir.dt.float32
AF = mybir.ActivationFunctionType
ALU = mybir.AluOpType
DR = mybir.MatmulPerfMode.DoubleRow

B, S, NLOC = 16, 512, 2048
FEAT, H, DH = 512, 8, 64
NCORES = 8
BL = B // NCORES          # batch per core = 2
R = BL * S                # rows per core = 1024
KT_N = NLOC // 128        # 16 contraction tiles for projections
FT_N = FEAT // 128        # 4 feature tiles
QT_N = S // 128           # 4 query tiles per batch element
NL_N = NLOC // 512        # 4 output column chunks
WS = 64.0                 # host weight scale for fp8 Q/K weights

_CACHE = {}


def _build(use_bias):
    nc = bacc.Bacc(
        "TRN2",
        target_bir_lowering=False,
        debug=False,
        enable_asserts=False,
        num_devices=NCORES,
    )

    # q/k pre-transposed and pre-tiled on host: [128, i*R + r] = x[r, i*128+p].
    d_q = nc.dram_tensor("q", [128, KT_N * R], FP8, kind="ExternalInput").ap()
    d_k = nc.dram_tensor("k", [128, KT_N * R], FP8, kind="ExternalInput").ap()
    d_v = nc.dram_tensor("v", [128, KT_N * R], BF16, kind="ExternalInput").ap()
    # esmT = exp(softmax(masked str_mat))^T, pre-tiled (computed on host):
    # [128, (b*4+kt)*512 + q] = exp(sm)[b, q, kt*128+p].
    d_sm = nc.dram_tensor("smh", [128, BL * QT_N * S], BF16, kind="ExternalInput").ap()
    # weights pre-tiled: wq/wk/wv [128, 16*512] with [p, i*512+f]=W.T[i*128+p, f];
    # wo [128, 4*2048] with [p, ft*2048+n]=Wo.T[ft*128+p, n].
    d_wq = nc.dram_tensor("wqT", [128, KT_N * FEAT], FP8, kind="ExternalInput").ap()
    d_wk = nc.dram_tensor("wkT", [128, KT_N * FEAT], FP8, kind="ExternalInput").ap()
    d_wv = nc.dram_tensor("wvT", [128, KT_N * FEAT], BF16, kind="ExternalInput").ap()
    d_wo = nc.dram_tensor("woT", [128, FT_N * NLOC], BF16, kind="ExternalInput").ap()
    d_bq = nc.dram_tensor("bqr", [1, FEAT], BF16, kind="ExternalInput").ap()
    d_bk = nc.dram_tensor("bkr", [1, FEAT], BF16, kind="ExternalInput").ap()
    d_bv = nc.dram_tensor("bvr", [1, FEAT], BF16, kind="ExternalInput").ap()
    d_bo = nc.dram_tensor("bor", [1, NLOC], BF16, kind="ExternalInput").ap()
    d_ones = nc.dram_tensor("onesr", [1, 512], BF16, kind="ExternalInput").ap()
    d_out = nc.dram_tensor("out", [R, NLOC], BF16, kind="ExternalOutput").ap()

    with tile.TileContext(nc) as tc:
        with (
            tc.tile_pool(name="consts", bufs=1) as cpool,
            tc.tile_pool(name="weights", bufs=1) as wpool,
            tc.tile_pool(name="persist", bufs=1) as ppool,
            tc.tile_pool(name="instream", bufs=6) as spool,
            tc.tile_pool(name="esmt", bufs=1) as mpool,
            tc.tile_pool(name="cols", bufs=3) as colpool,
            tc.tile_pool(name="attn", bufs=4) as apool,
            tc.tile_pool(name="ostage", bufs=2) as opool,
            tc.tile_pool(name="psum", bufs=4, space="PSUM") as psum,
        ):
            def dma_psplit(dst, src, parts=4, eng=None):
                """Issue a DMA as `parts` partition-range slices so it runs
                on several DMA queues in parallel (per-queue line rate is the
                bottleneck for multi-hundred-KB transfers)."""
                step = dst.shape[0] // parts
                for j in range(parts):
                    (eng or nc.sync).dma_start(
                        dst[j * step : (j + 1) * step], src[j * step : (j + 1) * step]
                    )

            ones = cpool.tile([1, 512], BF16, tag="ones", name="ones")
            biases = {}
            if use_bias:
                for nm, dr, width in (
                    ("bq", d_bq, FEAT),
                    ("bk", d_bk, FEAT),
                    ("bv", d_bv, FEAT),
                    ("bo", d_bo, NLOC),
                ):
                    t = cpool.tile([1, width], BF16, tag=nm, name=nm)
                    nc.sync.dma_start(t[:], dr[:])
                    biases[nm] = t

            # Persistent activations.
            QT = [ppool.tile([128, R], BF16, tag=f"QT{i}", name=f"QT{i}") for i in range(FT_N)]
            KTt = [ppool.tile([128, R], BF16, tag=f"KT{i}", name=f"KT{i}") for i in range(FT_N)]
            V6 = ppool.tile([128, R // 128, H, 2 * DH], BF16, tag="V6", name="V6")
            xT = [
                [ppool.tile([128, S], BF16, tag=f"xT{b}{j}", name=f"xT{b}{j}") for j in range(FT_N)]
                for b in range(BL)
            ]
            sm_t = [
                mpool.tile([128, QT_N * S], BF16, tag=f"smh{b}", name=f"smh{b}")
                for b in range(BL)
            ]

            wq = wpool.tile([128, KT_N, FEAT], FP8, tag="wq", name="wq")
            wk = wpool.tile([128, KT_N, FEAT], FP8, tag="wk", name="wk")
            wv = wpool.tile([128, KT_N, FEAT], BF16, tag="wv", name="wv")
            wo = wpool.tile([128, FT_N, NLOC], BF16, tag="wo", name="wo")

            def proj_dr(dst, d_src, w, d_w, scale, bias_nm, wchunks, first=False):
                """Q/K projection: fp8 DoubleRow, dst[ft][f, r] bf16 tiles.
                PSUM pair tile per ft covers both rc halves."""
                groups = {}
                for ft in range(FT_N):
                    ps = psum.tile([128, 2, 512], F32, tag="ps", name="ps")
                    if use_bias:
                        for rc in range(2):
                            nc.tensor.matmul(
                                ps[:, rc, :],
                                lhsT=biases[bias_nm][0:1, ft * 128 : (ft + 1) * 128],
                                rhs=ones[0:1, :],
                                start=True,
                                stop=False,
                            )
                    groups[ft] = ps
                per = (KT_N // 2) // wchunks
                for i in range(KT_N // 2):  # k-subtile pairs
                    if i % per == 0:
                        dma_psplit(
                            w[:, 2 * i : 2 * (i + per), :],
                            d_w[:, 2 * i * FEAT : 2 * (i + per) * FEAT],
                            parts=(4 if i == 0 and first else 1),
                        )
                    xt = spool.tile([128, 2, R], FP8, tag="xin", name="xin")
                    dma_psplit(
                        xt[:],
                        d_src[:, 2 * i * R : 2 * (i + 1) * R],
                        parts=(4 if i < 2 and first else 1),
                    )
                    for ft in range(FT_N):
                        for rc in range(2):
                            nc.tensor.matmul(
                                groups[ft][:, rc, :],
                                lhsT=w[:, 2 * i : 2 * i + 2, ft * 128 : (ft + 1) * 128],
                                rhs=xt[:, :, rc * 512 : (rc + 1) * 512],
                                start=(i == 0 and not use_bias),
                                stop=(i == KT_N // 2 - 1),
                                perf_mode=DR,
                            )
                for ft in range(FT_N):
                    nc.scalar.mul(dst[ft][:, 0:R], groups[ft][:, :, :], scale)

            nc.sync.dma_start(ones[:], d_ones[:])
            for b in range(BL):
                nc.sync.dma_start(
                    sm_t[b][:], d_sm[:, b * QT_N * S : (b + 1) * QT_N * S]
                )
            proj_dr(QT, d_q, wq, d_wq, 1.0 / (WS * DH), "bq", wchunks=4, first=True)
            proj_dr(KTt, d_k, wk, d_wk, 1.0 / WS, "bk", wchunks=2)
            dma_psplit(wv[:], d_wv[:])

            # V projection directly into [rows, feat] (no transpose needed):
            # V[r, f] = sum_nl v[r, nl] WvT[nl, f]; lhsT = v^T slice.
            vgroups = []
            for j in range(R // 256):
                ps = psum.tile([128, 2, 512], F32, tag="ps", name="ps")
                if use_bias:
                    for half in range(2):
                        nc.tensor.matmul(
                            ps[:, half, :],
                            lhsT=ones[0:1, 0:128],
                            rhs=biases["bv"][0:1, :],
                            start=True,
                            stop=False,
                        )
                vgroups.append(ps)
            for i in range(KT_N):
                vt = spool.tile([128, R], BF16, tag="vin", name="vin")
                nc.sync.dma_start(vt[:], d_v[:, i * R : (i + 1) * R])
                for rt in range(R // 128):
                    nc.tensor.matmul(
                        vgroups[rt // 2][:, rt % 2, :],
                        lhsT=vt[:, rt * 128 : (rt + 1) * 128],
                        rhs=wv[:, i, :],
                        start=(i == 0 and not use_bias),
                        stop=(i == KT_N - 1),
                    )
            # Block layout per head: [1...1 (64) | V_h (64)] -> PV rows 0-63
            # hold the replicated row-sum (offset-0 PSUM read for the fast
            # reciprocal), rows 64-127 hold y.
            nc.sync.dma_start(wo[:], d_wo[:])
            nc.vector.memset(V6[:, :, :, 0:DH], 1.0)
            for j in range(R // 256):
                nc.vector.tensor_copy(
                    V6[:, 2 * j : 2 * j + 2, :, DH : 2 * DH],
                    vgroups[j][:].rearrange("p t (h d) -> p t h d", h=H),
                )

            # ---- attention (transposed-scores flow) ---------------------
            for b in range(BL):
                for hp in range(H // 2):
                    ET = {
                        hs: apool.tile([128, QT_N, S], BF16, tag=f"ET{hs}", name=f"ET{hs}")
                        for hs in range(2)
                    }
                    spairs = {}
                    for hs in range(2):
                        hb = hs * 64
                        for kt in range(QT_N):
                            if kt % 2 == 0:
                                spairs[(hs, kt // 2)] = psum.tile(
                                    [128, 2, 512], F32, tag="ps", name="ps"
                                )
                            nc.tensor.matmul(
                                spairs[(hs, kt // 2)][:, kt % 2, :],
                                lhsT=KTt[hp][
                                    hb : hb + 64,
                                    b * S + kt * 128 : b * S + (kt + 1) * 128,
                                ],
                                rhs=QT[hp][hb : hb + 64, b * S : (b + 1) * S],
                                start=True,
                                stop=True,
                            )
                    for hs in range(2):
                        for j in range(2):
                            es = apool.tile([128, 2, S], BF16, tag="es", name="es")
                            nc.scalar.activation(
                                es[:], spairs[(hs, j)][:], AF.Exp
                            )
                            nc.vector.tensor_tensor(
                                ET[hs][:, 2 * j : 2 * j + 2, :],
                                es[:],
                                sm_t[b][:, 2 * j * S : (2 * j + 2) * S],
                                op=ALU.mult,
                            )
                    yp = psum.tile([128, 2, 512], F32, tag="ps", name="ps")
                    for hs in range(2):
                        h = 2 * hp + hs
                        for kt in range(QT_N):
                            nc.tensor.matmul(
                                yp[:, hs, :],
                                lhsT=V6[:, b * QT_N + kt, h, :],
                                rhs=ET[hs][:, kt, :],
                                start=(kt == 0),
                                stop=(kt == QT_N - 1),
                            )
                    rs2 = colpool.tile([64, 2, S], F32, tag="rs2", name="rs2")
                    nc.vector.reciprocal_approx_fast(
                        rs2[:], yp[0:DH, :, :]
                    )
                    for hs in range(2):
                        hb = hs * 64
                        nc.vector.tensor_tensor(
                            xT[b][hp][hb : hb + 64, :],
                            yp[DH : 2 * DH, hs, :],
                            rs2[:, hs, :],
                            op=ALU.mult,
                        )

            # ---- output projection (bf16 staging, then DMA) --------------
            for b in range(BL):
                for qt in range(QT_N):
                    row0 = b * S + qt * 128
                    ot = opool.tile([128, NLOC], BF16, tag="ot", name="ot")
                    for j in range(2):  # nlc pairs
                        ps = psum.tile([128, 2, 512], F32, tag="ps", name="ps")
                        for half in range(2):
                            nlc = 2 * j + half
                            if use_bias:
                                nc.tensor.matmul(
                                    ps[:, half, :],
                                    lhsT=ones[0:1, 0:128],
                                    rhs=biases["bo"][0:1, nlc * 512 : (nlc + 1) * 512],
                                    start=True,
                                    stop=False,
                                )
                            for ft in range(FT_N):
                                nc.tensor.matmul(
                                    ps[:, half, :],
                                    lhsT=xT[b][ft][:, qt * 128 : (qt + 1) * 128],
                                    rhs=wo[:, ft, nlc * 512 : (nlc + 1) * 512],
                                    start=(ft == 0 and not use_bias),
                                    stop=(ft == FT_N - 1),
                                )
                        dst = ot[:, 2 * j * 512 : (2 * j + 2) * 512]
                        if j == 0:
                            nc.scalar.copy(dst, ps[:, :, :])
                        else:
                            nc.vector.tensor_copy(dst, ps[:, :, :])
                        nc.sync.dma_start(
                            d_out[row0 : row0 + 128, 2 * j * 512 : (2 * j + 2) * 512],
                            dst,
                        )

    nc.compile()
    return nc


def _prep_inputs(q, k, v, str_mat, attn_mask, Wq, bq, Wk, bk, Wv, bv, Wo, bo):
    bf = ml_dtypes.bfloat16
    f8 = ml_dtypes.float8_e4m3
    # fp8 Q/K weights host-scaled by WS=64 to stay in normal range; the
    # PSUM copy-out divides it back (and folds 1/DH for Q).
    wqT = np.ascontiguousarray((Wq * np.float32(WS)).T).astype(f8)
    wkT = np.ascontiguousarray((Wk * np.float32(WS)).T).astype(f8)
    wvT = np.ascontiguousarray(Wv.T).astype(bf)
    woT = np.ascontiguousarray(Wo.T).astype(bf)

    # Pre-tile weights: [n*128, width] -> [128, n*width].
    def pretile(w):
        n = w.shape[0] // 128
        return np.ascontiguousarray(
            w.reshape(n, 128, w.shape[1]).transpose(1, 0, 2).reshape(128, -1)
        )

    wqt = pretile(wqT)
    wkt = pretile(wkT)
    wvt = pretile(wvT)
    wot = pretile(woT)

    bqr = (bq[None, :] * np.float32(WS / DH)).astype(bf)
    bkr = (bk[None, :] * np.float32(WS)).astype(bf)
    bvr = bv[None, :].astype(bf)
    bor = bo[None, :].astype(bf)
    onesr = np.ones((1, 512), dtype=bf)

    q8 = np.asarray(q).astype(f8)
    k8 = np.asarray(k).astype(f8)
    v16 = np.asarray(v).astype(bf)

    def pretile_T(x):
        # [R, NLOC] -> [128, KT_N*R] with [p, i*R+r] = x[r, i*128+p]
        return np.ascontiguousarray(
            x.reshape(R, KT_N, 128).transpose(2, 1, 0).reshape(128, KT_N * R)
        )

    # Structural softmax on host; upload exp of its TRANSPOSE in bf16.
    strf = np.asarray(str_mat, dtype=np.float32)
    maskf = np.asarray(attn_mask)
    sm = np.where(maskf == 0, np.float32(-1e9), strf)
    sm = sm - sm.max(-1, keepdims=True)
    np.exp(sm, out=sm)
    sm /= sm.sum(-1, keepdims=True)
    smT16 = np.exp(np.ascontiguousarray(sm.transpose(0, 2, 1))).astype(bf)

    in_maps = []
    for c in range(NCORES):
        sl = slice(c * BL, (c + 1) * BL)
        # [BL, S(k), S(q)] -> [128, BL*QT_N*S] with [p, (b*4+kt)*S+q].
        smt = np.ascontiguousarray(
            smT16[sl].reshape(BL * QT_N, 128, S).transpose(1, 0, 2).reshape(128, -1)
        )
        in_maps.append(
            {
                "q": pretile_T(q8[sl].reshape(R, NLOC)),
                "k": pretile_T(k8[sl].reshape(R, NLOC)),
                "v": pretile_T(v16[sl].reshape(R, NLOC)),
                "smh": smt,
                "wqT": wqt,
                "wkT": wkt,
                "wvT": wvt,
                "woT": wot,
                "bqr": bqr,
                "bkr": bkr,
                "bvr": bvr,
                "bor": bor,
                "onesr": onesr,
            }
        )
    return in_maps


def kernel(q, k, v, str_mat, attn_mask, Wq, bq, Wk, bk, Wv, bv, Wo, bo):
    use_bias = bool(
        np.any(np.asarray(bq))
        or np.any(np.asarray(bk))
        or np.any(np.asarray(bv))
        or np.any(np.asarray(bo))
    )
    key = ("nc", use_bias)
    if key not in _CACHE:
        _CACHE[key] = _build(use_bias)
    nc = _CACHE[key]
    in_maps = _prep_inputs(
        q, k, v, str_mat, attn_mask, Wq, bq, Wk, bk, Wv, bv, Wo, bo
    )
    res = run_bass_kernel_spmd(nc, in_maps, core_ids=list(range(NCORES)))
    out = np.empty((B, S, NLOC), dtype=np.float32)
    for c in range(NCORES):
        out[c * BL : (c + 1) * BL] = (
            res.results[c]["out"].astype(np.float32).reshape(BL, S, NLOC)
        )
    return out


# revision 60
# speedup vs baseline: 1.0314x; 1.0314x over previous
"""Trainium2 Bass kernel for nn_Attn_17738214933129.

Dense transformer attention block:
  Q/K/V projections from n_loc=2048 -> feat=512 (8 heads x 64),
  structural-bias softmax added to scaled QK^T scores, softmax, PV,
  output projection back to n_loc=2048.

Sharding: data-parallel over batch (16 -> 2 per core) across 8 NeuronCores,
weights replicated, no collectives.

Layout strategy (per core, rows = 2*512 = 1024):
  - q/k are uploaded pre-transposed/pre-tiled in fp8e4; Q/K projections run
    as fp8 DoubleRow matmuls (2 k-subtiles per instruction, 2x PE rate)
    with weights host-scaled by 64 to stay in fp8 normal range; the
    PSUM->SBUF copy descales (and folds the 1/DH for Q).
  - v is bf16; V is projected directly into [rows, feat] layout (input
    slice as the stationary operand), so no PE transpose pass is needed.
  - Attention uses the TRANSPOSED score layout: ST[k, q] = (QK^T)^T per
    head, so exp(ST)*esmT feeds the PV matmul directly (contraction over
    keys on partitions) -- no on-chip transposes at all.  The structural
    softmax is precomputed on host and uploaded as esmT = exp(sm^T) bf16;
    the multiply runs on DVE in fast 2-byte mode.
  - V carries 64 ones-columns per head ([V_h | 1...1]), so PV output rows
    0-63 hold the softmax row-sum replicated 64x: the reciprocal runs wide
    on DVE and its output is already broadcast for the normalization,
    which fuses into the PSUM->SBUF copy as a tensor-tensor multiply.
  - All PSUM work uses 2-bank pair tiles [128, 2, 512] so elementwise ops
    (exp / esm-mult / copies / reciprocal) amortize their fixed per-op
    overhead over 1024 columns.
"""

import sys

import numpy as np

try:
    import concourse.bass as bass  # noqa: F401
except Exception:  # pragma: no cover - path fallback
    sys.path.insert(0, "/opt/trn_rl_repo")

import ml_dtypes

import concourse.bacc as bacc
import concourse.tile as tile
from concourse import mybir
from concourse.bass_utils import run_bass_kernel_spmd

BF16 = mybir.dt.bfloat16
FP8 = mybir.dt.float8e4
F32 = mybir.dt.float32
AF = mybir.ActivationFunctionType
ALU = mybir.AluOpType
DR = mybir.MatmulPerfMode.DoubleRow

B, S, NLOC = 16, 512, 2048
FEAT, H, DH = 512, 8, 64
NCORES = 8
BL = B // NCORES          # batch per core = 2
R = BL * S                # rows per core = 1024
KT_N = NLOC // 128        # 16 contraction tiles for projections
FT_N = FEAT // 128        # 4 feature tiles
QT_N = S // 128           # 4 query tiles per batch element
NL_N = NLOC // 512        # 4 output column chunks
WS = 64.0                 # host weight scale for fp8 Q/K weights

_CACHE = {}


def _build(use_bias):
    nc = bacc.Bacc(
        "TRN2",
        target_bir_lowering=False,
        debug=False,
        enable_asserts=False,
        num_devices=NCORES,
    )

    # q/k pre-transposed and pre-tiled on host: [128, i*R + r] = x[r, i*128+p].
    d_q = nc.dram_tensor("q", [128, KT_N * R], FP8, kind="ExternalInput").ap()
    d_k = nc.dram_tensor("k", [128, KT_N * R], FP8, kind="ExternalInput").ap()
    d_v = nc.dram_tensor("v", [128, KT_N * R], BF16, kind="ExternalInput").ap()
    # esmT = exp(softmax(masked str_mat))^T, pre-tiled (computed on host):
    # [128, (b*4+kt)*512 + q] = exp(sm)[b, q, kt*128+p].
    d_sm = nc.dram_tensor("smh", [128, BL * QT_N * S], BF16, kind="ExternalInput").ap()
    # weights pre-tiled: wq/wk/wv [128, 16*512] with [p, i*512+f]=W.T[i*128+p, f];
    # wo [128, 4*2048] with [p, ft*2048+n]=Wo.T[ft*128+p, n].
    d_wq = nc.dram_tensor("wqT", [128, KT_N * FEAT], FP8, kind="ExternalInput").ap()
    d_wk = nc.dram_tensor("wkT", [128, KT_N * FEAT], FP8, kind="ExternalInput").ap()
    d_wv = nc.dram_tensor("wvT", [128, KT_N * FEAT], BF16, kind="ExternalInput").ap()
    d_wo = nc.dram_tensor("woT", [128, FT_N * NLOC], BF16, kind="ExternalInput").ap()
    d_bq = nc.dram_tensor("bqr", [1, FEAT], BF16, kind="ExternalInput").ap()
    d_bk = nc.dram_tensor("bkr", [1, FEAT], BF16, kind="ExternalInput").ap()
    d_bv = nc.dram_tensor("bvr", [1, FEAT], BF16, kind="ExternalInput").ap()
    d_bo = nc.dram_tensor("bor", [1, NLOC], BF16, kind="ExternalInput").ap()
    d_ones = nc.dram_tensor("onesr", [1, 512], BF16, kind="ExternalInput").ap()
    d_out = nc.dram_tensor("out", [R, NLOC], BF16, kind="ExternalOutput").ap()

    with tile.TileContext(nc) as tc:
        with (
            tc.tile_pool(name="consts", bufs=1) as cpool,
            tc.tile_pool(name="weights", bufs=1) as wpool,
            tc.tile_pool(name="persist", bufs=1) as ppool,
            tc.tile_pool(name="instream", bufs=6) as spool,
            tc.tile_pool(name="esmt", bufs=1) as mpool,
            tc.tile_pool(name="cols", bufs=3) as colpool,
            tc.tile_pool(name="attn", bufs=3) as apool,
            tc.tile_pool(name="ostage", bufs=2) as opool,
            tc.tile_pool(name="psum", bufs=4, space="PSUM") as psum,
        ):
            def dma_psplit(dst, src, parts=4, eng=None):
                """Issue a DMA as `parts` partition-range slices so it runs
                on several DMA queues in parallel (per-queue line rate is the
                bottleneck for multi-hundred-KB transfers)."""
                step = dst.shape[0] // parts
                for j in range(parts):
                    (eng or nc.sync).dma_start(
                        dst[j * step : (j + 1) * step], src[j * step : (j + 1) * step]
                    )

            ones = cpool.tile([1, 512], BF16, tag="ones", name="ones")
            biases = {}
            if use_bias:
                for nm, dr, width in (
                    ("bq", d_bq, FEAT),
                    ("bk", d_bk, FEAT),
                    ("bv", d_bv, FEAT),
                    ("bo", d_bo, NLOC),
                ):
                    t = cpool.tile([1, width], BF16, tag=nm, name=nm)
                    nc.sync.dma_start(t[:], dr[:])
                    biases[nm] = t

            # Persistent activations.
            QT = [ppool.tile([128, R], BF16, tag=f"QT{i}", name=f"QT{i}") for i in range(FT_N)]
            KTt = [ppool.tile([128, R], BF16, tag=f"KT{i}", name=f"KT{i}") for i in range(FT_N)]
            V6 = ppool.tile([128, R // 128, H, 2 * DH], BF16, tag="V6", name="V6")
            xT = [
                [ppool.tile([128, S], BF16, tag=f"xT{b}{j}", name=f"xT{b}{j}") for j in range(FT_N)]
                for b in range(BL)
            ]
            sm_t = [
                mpool.tile([128, QT_N * S], BF16, tag=f"smh{b}", name=f"smh{b}")
                for b in range(BL)
            ]

            wq = wpool.tile([128, KT_N, FEAT], FP8, tag="wq", name="wq")
            wk = wpool.tile([128, KT_N, FEAT], FP8, tag="wk", name="wk")
            wv = wpool.tile([128, KT_N, FEAT], BF16, tag="wv", name="wv")
            wo = wpool.tile([128, FT_N, NLOC], BF16, tag="wo", name="wo")

            def proj_dr(dst, d_src, w, d_w, scale, bias_nm, wchunks, first=False):
                """Q/K projection: fp8 DoubleRow, dst[ft][f, r] bf16 tiles.
                PSUM pair tile per ft covers both rc halves."""
                groups = {}
                for ft in range(FT_N):
                    ps = psum.tile([128, 2, 512], F32, tag="ps", name="ps")
                    if use_bias:
                        for rc in range(2):
                            nc.tensor.matmul(
                                ps[:, rc, :],
                                lhsT=biases[bias_nm][0:1, ft * 128 : (ft + 1) * 128],
                                rhs=ones[0:1, :],
                                start=True,
                                stop=False,
                            )
                    groups[ft] = ps
                per = (KT_N // 2) // wchunks
                for i in range(KT_N // 2):  # k-subtile pairs
                    if i % per == 0:
                        dma_psplit(
                            w[:, 2 * i : 2 * (i + per), :],
                            d_w[:, 2 * i * FEAT : 2 * (i + per) * FEAT],
                            parts=(4 if i == 0 and first else 1),
                            eng=(nc.scalar if i == 0 and first else None),
                        )
                    xt = spool.tile([128, 2, R], FP8, tag="xin", name="xin")
                    dma_psplit(
                        xt[:],
                        d_src[:, 2 * i * R : 2 * (i + 1) * R],
                        parts=(4 if i < 2 and first else 1),
                    )
                    for ft in range(FT_N):
                        for rc in range(2):
                            nc.tensor.matmul(
                                groups[ft][:, rc, :],
                                lhsT=w[:, 2 * i : 2 * i + 2, ft * 128 : (ft + 1) * 128],
                                rhs=xt[:, :, rc * 512 : (rc + 1) * 512],
                                start=(i == 0 and not use_bias),
                                stop=(i == KT_N // 2 - 1),
                                perf_mode=DR,
                            )
                for ft in range(FT_N):
                    nc.scalar.mul(dst[ft][:, 0:R], groups[ft][:, :, :], scale)

            proj_dr(QT, d_q, wq, d_wq, 1.0 / (WS * DH), "bq", wchunks=4, first=True)
            nc.sync.dma_start(ones[:], d_ones[:])
            for b in range(BL):
                nc.sync.dma_start(
                    sm_t[b][:], d_sm[:, b * QT_N * S : (b + 1) * QT_N * S]
                )
            proj_dr(KTt, d_k, wk, d_wk, 1.0 / WS, "bk", wchunks=2)
            nc.sync.dma_start(wv[:], d_wv[:])

            # V projection directly into [rows, feat] (no transpose needed):
            # V[r, f] = sum_nl v[r, nl] WvT[nl, f]; lhsT = v^T slice.
            vgroups = []
            for j in range(R // 256):
                ps = psum.tile([128, 2, 512], F32, tag="ps", name="ps")
                if use_bias:
                    for half in range(2):
                        nc.tensor.matmul(
                            ps[:, half, :],
                            lhsT=ones[0:1, 0:128],
                            rhs=biases["bv"][0:1, :],
                            start=True,
                            stop=False,
                        )
                vgroups.append(ps)
            for i in range(KT_N):
                vt = spool.tile([128, R], BF16, tag="vin", name="vin")
                nc.sync.dma_start(vt[:], d_v[:, i * R : (i + 1) * R])
                for rt in range(R // 128):
                    nc.tensor.matmul(
                        vgroups[rt // 2][:, rt % 2, :],
                        lhsT=vt[:, rt * 128 : (rt + 1) * 128],
                        rhs=wv[:, i, :],
                        start=(i == 0 and not use_bias),
                        stop=(i == KT_N - 1),
                    )
            # Block layout per head: [1...1 (64) | V_h (64)] -> PV rows 0-63
            # hold the replicated row-sum (offset-0 PSUM read for the fast
            # reciprocal), rows 64-127 hold y.
            nc.sync.dma_start(wo[:], d_wo[:])
            nc.vector.memset(V6[:, :, :, 0:DH], 1.0)
            for j in range(R // 256):
                nc.vector.tensor_copy(
                    V6[:, 2 * j : 2 * j + 2, :, DH : 2 * DH],
                    vgroups[j][:].rearrange("p t (h d) -> p t h d", h=H),
                )

            # ---- attention (transposed-scores flow) ---------------------
            for b in range(BL):
                for hp in range(H // 2):
                    ET = {
                        hs: apool.tile([128, QT_N, S], BF16, tag=f"ET{hs}", name=f"ET{hs}")
                        for hs in range(2)
                    }
                    spairs = {}
                    for hs in range(2):
                        hb = hs * 64
                        for kt in range(QT_N):
                            if kt % 2 == 0:
                                spairs[(hs, kt // 2)] = psum.tile(
                                    [128, 2, 512], F32, tag="ps", name="ps"
                                )
                            nc.tensor.matmul(
                                spairs[(hs, kt // 2)][:, kt % 2, :],
                                lhsT=KTt[hp][
                                    hb : hb + 64,
                                    b * S + kt * 128 : b * S + (kt + 1) * 128,
                                ],
                                rhs=QT[hp][hb : hb + 64, b * S : (b + 1) * S],
                                start=True,
                                stop=True,
                            )
                    for hs in range(2):
                        for j in range(2):
                            es = apool.tile([128, 2, S], BF16, tag="es", name="es")
                            nc.scalar.activation(
                                es[:], spairs[(hs, j)][:], AF.Exp
                            )
                            nc.vector.tensor_tensor(
                                ET[hs][:, 2 * j : 2 * j + 2, :],
                                es[:],
                                sm_t[b][:, 2 * j * S : (2 * j + 2) * S],
                                op=ALU.mult,
                            )
                    yp = psum.tile([128, 2, 512], F32, tag="ps", name="ps")
                    for hs in range(2):
                        h = 2 * hp + hs
                        for kt in range(QT_N):
                            nc.tensor.matmul(
                                yp[:, hs, :],
                                lhsT=V6[:, b * QT_N + kt, h, :],
                                rhs=ET[hs][:, kt, :],
                                start=(kt == 0),
                                stop=(kt == QT_N - 1),
                            )
                    rs2 = colpool.tile([64, 2, S], F32, tag="rs2", name="rs2")
                    nc.vector.reciprocal_approx_fast(
                        rs2[:], yp[0:DH, :, :]
                    )
                    for hs in range(2):
                        hb = hs * 64
                        nc.vector.tensor_tensor(
                            xT[b][hp][hb : hb + 64, :],
                            yp[DH : 2 * DH, hs, :],
                            rs2[:, hs, :],
                            op=ALU.mult,
                        )

            # ---- output projection (bf16 staging, then DMA) --------------
            for b in range(BL):
                for qt in range(QT_N):
                    row0 = b * S + qt * 128
                    ot = opool.tile([128, NLOC], BF16, tag="ot", name="ot")
                    for j in range(2):  # nlc pairs
                        ps = psum.tile([128, 2, 512], F32, tag="ps", name="ps")
                        for half in range(2):
                            nlc = 2 * j + half
                            if use_bias:
                                nc.tensor.matmul(
                                    ps[:, half, :],
                                    lhsT=ones[0:1, 0:128],
                                    rhs=biases["bo"][0:1, nlc * 512 : (nlc + 1) * 512],
                                    start=True,
                                    stop=False,
                                )
                            for ft in range(FT_N):
                                nc.tensor.matmul(
                                    ps[:, half, :],
                                    lhsT=xT[b][ft][:, qt * 128 : (qt + 1) * 128],
                                    rhs=wo[:, ft, nlc * 512 : (nlc + 1) * 512],
                                    start=(ft == 0 and not use_bias),
                                    stop=(ft == FT_N - 1),
                                )
                        dst = ot[:, 2 * j * 512 : (2 * j + 2) * 512]
                        if j == 0:
                            nc.scalar.copy(dst, ps[:, :, :])
                        else:
                            nc.vector.tensor_copy(dst, ps[:, :, :])
                        nc.sync.dma_start(
                            d_out[row0 : row0 + 128, 2 * j * 512 : (2 * j + 2) * 512],
                            dst,
                        )

    nc.compile()
    return nc


def _prep_inputs(q, k, v, str_mat, attn_mask, Wq, bq, Wk, bk, Wv, bv, Wo, bo):
    bf = ml_dtypes.bfloat16
    f8 = ml_dtypes.float8_e4m3
    # fp8 Q/K weights host-scaled by WS=64 to stay in normal range; the
    # PSUM copy-out divides it back (and folds 1/DH for Q).
    wqT = np.ascontiguousarray((Wq * np.float32(WS)).T).astype(f8)
    wkT = np.ascontiguousarray((Wk * np.float32(WS)).T).astype(f8)
    wvT = np.ascontiguousarray(Wv.T).astype(bf)
    woT = np.ascontiguousarray(Wo.T).astype(bf)

    # Pre-tile weights: [n*128, width] -> [128, n*width].
    def pretile(w):
        n = w.shape[0] // 128
        return np.ascontiguousarray(
            w.reshape(n, 128, w.shape[1]).transpose(1, 0, 2).reshape(128, -1)
        )

    wqt = pretile(wqT)
    wkt = pretile(wkT)
    wvt = pretile(wvT)
    wot = pretile(woT)

    bqr = (bq[None, :] * np.float32(WS / DH)).astype(bf)
    bkr = (bk[None, :] * np.float32(WS)).astype(bf)
    bvr = bv[None, :].astype(bf)
    bor = bo[None, :].astype(bf)
    onesr = np.ones((1, 512), dtype=bf)

    q8 = np.asarray(q).astype(f8)
    k8 = np.asarray(k).astype(f8)
    v16 = np.asarray(v).astype(bf)

    def pretile_T(x):
        # [R, NLOC] -> [128, KT_N*R] with [p, i*R+r] = x[r, i*128+p]
        return np.ascontiguousarray(
            x.reshape(R, KT_N, 128).transpose(2, 1, 0).reshape(128, KT_N * R)
        )

    # Structural softmax on host; upload exp of its TRANSPOSE in bf16.
    strf = np.asarray(str_mat, dtype=np.float32)
    maskf = np.asarray(attn_mask)
    sm = np.where(maskf == 0, np.float32(-1e9), strf)
    sm = sm - sm.max(-1, keepdims=True)
    np.exp(sm, out=sm)
    sm /= sm.sum(-1, keepdims=True)
    smT16 = np.exp(np.ascontiguousarray(sm.transpose(0, 2, 1))).astype(bf)

    in_maps = []
    for c in range(NCORES):
        sl = slice(c * BL, (c + 1) * BL)
        # [BL, S(k), S(q)] -> [128, BL*QT_N*S] with [p, (b*4+kt)*S+q].
        smt = np.ascontiguousarray(
            smT16[sl].reshape(BL * QT_N, 128, S).transpose(1, 0, 2).reshape(128, -1)
        )
        in_maps.append(
            {
                "q": pretile_T(q8[sl].reshape(R, NLOC)),
                "k": pretile_T(k8[sl].reshape(R, NLOC)),
                "v": pretile_T(v16[sl].reshape(R, NLOC)),
                "smh": smt,
                "wqT": wqt,
                "wkT": wkt,
                "wvT": wvt,
                "woT": wot,
                "bqr": bqr,
                "bkr": bkr,
                "bvr": bvr,
                "bor": bor,
                "onesr": onesr,
            }
        )
    return in_maps


def kernel(q, k, v, str_mat, attn_mask, Wq, bq, Wk, bk, Wv, bv, Wo, bo):
    use_bias = bool(
        np.any(np.asarray(bq))
        or np.any(np.asarray(bk))
        or np.any(np.asarray(bv))
        or np.any(np.asarray(bo))
    )
    key = ("nc", use_bias)
    if key not in _CACHE:
        _CACHE[key] = _build(use_bias)
    nc = _CACHE[key]
    in_maps = _prep_inputs(
        q, k, v, str_mat, attn_mask, Wq, bq, Wk, bk, Wv, bv, Wo, bo
    )
    res = run_bass_kernel_spmd(nc, in_maps, core_ids=list(range(NCORES)))
    out = np.empty((B, S, NLOC), dtype=np.float32)
    for c in range(NCORES):
        out[c * BL : (c + 1) * BL] = (
            res.results[c]["out"].astype(np.float32).reshape(BL, S, NLOC)
        )
    return out
